# revision 29
# baseline (speedup 1.0000x reference)
"""TRN2 Bass kernel for GPT-style causal self-attention with RoPE (bf16).

Reference (B=2, S=2048, D=1024, H=16, dk=64):
  qkv = hidden @ c_attn_w + c_attn_b; rope(q), rope(k) via position_ids;
  out = softmax(causal(q k^T / 8)) v, merged heads, @ c_proj_w + c_proj_b.

Sharding across 8 NeuronCores: core c = 4*b + g handles batch b and head
group g (4 heads = 256 dims). Each core computes its full S x S attention
and a row-sliced c_proj partial; the host sums the 4 partials per batch.

Kernel structure (all matmul operands bf16, fp32 PSUM accumulation):
  1. q^T/k^T computed directly in transposed layout (w chunks stationary,
     hT moving), with head dims pair-interleaved (rows 2i/2i+1 = dims
     i/i+32) so the rope partner sits on the adjacent partition; rope =
     stream_shuffle + 2 muls + add on DVE against host-prepped cosT/sinT.
     v computed in natural [s, d] layout (hT stationary), ones column
     appended for softmax denominators.
  2. Per head-pair, per 512-wide q chunk: scores^T via row-tiled K=64
     matmul pairs (two heads concurrent in the PE); exp (scale 1/8) on
     ScalarE -> bf16 P; causal diagonal 0/1 mask on GPSIMD post-exp;
     PV: out[0:65] = [v | ones]^T @ P^T accumulated over k blocks (row 64
     = denominators); normalize via reciprocal_approx_fast from PSUM +
     K=1 PE broadcast matmul + DVE multiply.
  3. projT[d, s] = Wp_chunk^T @ attn^T + bias (ACT), bf16 out DMA.
Stages are emitted interleaved (sg0, sg1, c0, sg2, c1, p0, ...) so the PE
stays dense (HAM stays warm); warmup matmuls cover the initial hT DMA.
"""

from contextlib import ExitStack

import numpy as np

import concourse.bacc as bacc
import concourse.tile as tile
import concourse.mybir as mybir
from concourse.bass_utils import run_bass_kernel_spmd

f32 = mybir.dt.float32
f32r = mybir.dt.float32r
bf16 = mybir.dt.bfloat16
AF = mybir.ActivationFunctionType
ALU = mybir.AluOpType

S = 2048
D = 1024
HD = 256           # head dims per core (4 heads x 64)
SB = S // 128      # 16
KC = D // 128      # 8
NCH = S // 512     # 4
SWAP_MASK = [i ^ 1 for i in range(32)]  # pair swap within quadrant


def build_attention_nc(num_devices=8):
    nc = bacc.Bacc("TRN2", target_bir_lowering=False, debug=False,
                   num_devices=num_devices)

    hT_d = nc.dram_tensor("hT", [D, S], bf16, kind="ExternalInput")
    wqkv_d = nc.dram_tensor("wqkv", [D, 768], bf16, kind="ExternalInput")
    bqk_d = nc.dram_tensor("bqk", [128, 4], f32, kind="ExternalInput")
    bv_d = nc.dram_tensor("bv", [1, 256], bf16, kind="ExternalInput")
    bpT_d = nc.dram_tensor("bpT", [1, D], bf16, kind="ExternalInput")
    cosT_d = nc.dram_tensor("cosT", [128, S], bf16, kind="ExternalInput")
    sinT_d = nc.dram_tensor("sinT", [128, S], bf16, kind="ExternalInput")
    wp_d = nc.dram_tensor("wp", [HD, D], bf16, kind="ExternalInput")
    bp_d = nc.dram_tensor("bp", [128, 8], f32, kind="ExternalInput")
    mask2_d = nc.dram_tensor("mask2", [128, 256], bf16, kind="ExternalInput")
    ones64_d = nc.dram_tensor("ones64", [128, 64], bf16, kind="ExternalInput")
    onesrow_d = nc.dram_tensor("ones_row", [1, 512], bf16, kind="ExternalInput")
    outT_d = nc.dram_tensor("outT", [D, S], bf16, kind="ExternalOutput")
    warm_d = nc.dram_tensor("warm", [128, 512], bf16, kind="ExternalOutput")

    with tile.TileContext(nc) as tc, ExitStack() as top:
        const = top.enter_context(tc.tile_pool(name="const", bufs=1))
        persist = top.enter_context(tc.tile_pool(name="persist", bufs=1))

        # batched DMAs, critical-path first: hT(sg0), w, consts, rest
        hT_sb = persist.tile([128, KC, S], bf16, tag="hT")
        hT_src = hT_d.ap().rearrange("(a p) s -> p a s", a=KC)
        nc.sync.dma_start(hT_sb[:, :, 0:512], hT_src[:, :, 0:512])
        w_sb = persist.tile([128, KC, 768], bf16, tag="w")
        nc.sync.dma_start(w_sb[:],
                          wqkv_d.ap().rearrange("(a p) j -> p a j", a=KC))

        mask2 = const.tile([128, 256], bf16, tag="mask2")
        nc.sync.dma_start(mask2[:], mask2_d.ap())
        ones_row = const.tile([1, 512], bf16, tag="ones_row")
        nc.sync.dma_start(ones_row[:], onesrow_d.ap())
        ones64 = const.tile([128, 64], bf16, tag="ones64")
        nc.sync.dma_start(ones64[:], ones64_d.ap())
        bqk_sb = const.tile([128, 4], f32, tag="bqk")
        nc.sync.dma_start(bqk_sb[:], bqk_d.ap())
        bv_sb = const.tile([1, 256], bf16, tag="bv")
        nc.sync.dma_start(bv_sb[:], bv_d.ap())
        bp_sb = const.tile([128, 8], f32, tag="bp")
        nc.sync.dma_start(bp_sb[:], bp_d.ap())
        bpT_sb = const.tile([1, D], bf16, tag="bpT")
        nc.sync.dma_start(bpT_sb[:], bpT_d.ap())

        nc.sync.dma_start(hT_sb[:, :, 512:1024], hT_src[:, :, 512:1024])
        cosT_sb = persist.tile([128, S], bf16, tag="cosT")
        nc.sync.dma_start(cosT_sb[:], cosT_d.ap())
        sinT_sb = persist.tile([128, S], bf16, tag="sinT")
        nc.sync.dma_start(sinT_sb[:], sinT_d.ap())
        for sg in range(2, NCH):
            ssl = slice(sg * 512, (sg + 1) * 512)
            nc.sync.dma_start(hT_sb[:, :, ssl], hT_src[:, :, ssl])
        wp_sb = persist.tile([128, 2, D], bf16, tag="wp")
        nc.sync.dma_start(wp_sb[:],
                          wp_d.ap().rearrange("(a p) j -> p a j", a=2))

        qT = [persist.tile([128, S], bf16, tag=f"qT{hp}", name=f"qT{hp}")
              for hp in range(2)]
        kT = [persist.tile([128, S], bf16, tag=f"kT{hp}", name=f"kT{hp}")
              for hp in range(2)]
        v_sb = persist.tile([128, SB, 4, 66], bf16, tag="v")
        nc.scalar.copy(v_sb[:, :, :, 64],
                       ones64[:].rearrange("p (a b) -> p a b", a=SB))
        aT = [persist.tile([128, S], bf16, tag=f"aT{hp}", name=f"aT{hp}")
              for hp in range(2)]
        aTo = [persist.tile([64, S], bf16, tag=f"aTo{hp}", name=f"aTo{hp}")
               for hp in range(2)]

        # PSUM pools: 4 + 4 = 8 banks
        mm512 = top.enter_context(tc.tile_pool(name="mm512", bufs=4,
                                               space="PSUM"))
        stps = top.enter_context(tc.tile_pool(name="stps", bufs=2,
                                              space="PSUM"))
        # SBUF working pools
        work = top.enter_context(tc.tile_pool(name="work", bufs=2))
        pt_pool = top.enter_context(tc.tile_pool(name="pt", bufs=20))
        nrm = top.enter_context(tc.tile_pool(name="nrm", bufs=2))
        po_pool = top.enter_context(tc.tile_pool(name="po", bufs=2))

        # ---- warmup: keep PE busy during initial DMA; preload exp table ----
        junk = const.tile([128, 512], bf16, tag="junk")
        nc.vector.memset(junk[:], 0.0)
        junk_e = const.tile([1, 16], bf16, tag="junk_e")
        nc.scalar.activation(junk_e[:], junk[0:1, 0:16], AF.Exp, scale=0.125)
        warm_ps = mm512.tile([128, 512], f32, tag="mm512")
        for i in range(28):
            nc.tensor.matmul(warm_ps[:], junk[:, 0:128], junk[:],
                             start=(i == 0), stop=(i == 27))
        warm_sb = const.tile([128, 512], bf16, tag="warm_sb")
        nc.vector.tensor_copy(warm_sb[:], warm_ps[:])
        nc.sync.dma_start(warm_d.ap(), warm_sb[:])

        def stage1(sg):
            ssl = slice(sg * 512, (sg + 1) * 512)
            for jc in range(4):
                acc = mm512.tile([128, 512], f32, tag="mm512")
                for dc in range(KC):
                    nc.tensor.matmul(acc[:],
                                     w_sb[:, dc, jc * 128:(jc + 1) * 128],
                                     hT_sb[:, dc, ssl],
                                     start=(dc == 0), stop=(dc == KC - 1))
                raw = work.tile([128, 512], bf16, tag="raw")
                nc.scalar.activation(raw[:], acc[:], AF.Identity,
                                     bias=bqk_sb[:, jc:jc + 1])
                shuf = work.tile([128, 512], bf16, tag="shuf")
                nc.vector.stream_shuffle(shuf[:], raw[:], mask=SWAP_MASK)
                m1 = work.tile([128, 512], bf16, tag="m1")
                nc.vector.tensor_tensor(m1[:], raw[:], cosT_sb[:, ssl],
                                        op=ALU.mult)
                dest = (qT if jc < 2 else kT)[jc % 2]
                m2 = work.tile([128, 512], bf16, tag="m2")
                nc.vector.tensor_tensor(m2[:], shuf[:], sinT_sb[:, ssl],
                                        op=ALU.mult)
                nc.vector.tensor_tensor(dest[:, ssl], m1[:], m2[:],
                                        op=ALU.add)
            for sbl in range(4):
                sb = sg * 4 + sbl
                vp = mm512.tile([128, 256], f32, tag="mm512")
                for dc in range(KC):
                    nc.tensor.matmul(vp[:],
                                     hT_sb[:, dc, sb * 128:(sb + 1) * 128],
                                     w_sb[:, dc, 512:768],
                                     start=(dc == 0), stop=False)
                nc.tensor.matmul(vp[:], ones_row[:, 0:128], bv_sb[:],
                                 start=False, stop=True)
                nc.vector.tensor_copy(
                    v_sb[:, sb, :, 0:64],
                    vp[:].rearrange("p (h d) -> p h d", h=4))

        def stage2(c):
            csl = slice(c * 512, (c + 1) * 512)
            nkb = 4 * c + 4
            for hp in range(2):
                pts = []
                for kb in range(nkb):
                    q0 = max(512 * c, 128 * kb)
                    off = q0 - 512 * c
                    st_p = stps.tile([128, 2, 512], f32, tag="st")
                    for h2 in range(2):
                        nc.tensor.matmul(
                            st_p[:, h2, off:512],
                            kT[hp][h2 * 64:(h2 + 1) * 64,
                                   kb * 128:(kb + 1) * 128],
                            qT[hp][h2 * 64:(h2 + 1) * 64,
                                   q0:512 * (c + 1)],
                            start=True, stop=True,
                            tile_position=(h2 * 64, 0))
                    pt = pt_pool.tile([128, 2, 512], bf16, tag="pt")
                    nc.scalar.activation(pt[:, :, off:512],
                                         st_p[:, :, off:512],
                                         AF.Exp, scale=0.125)
                    if 128 * kb >= 512 * c:
                        nc.vector.tensor_tensor(
                            pt[:, :, off:off + 128],
                            pt[:, :, off:off + 128],
                            mask2[:].rearrange("p (a b) -> p a b", a=2),
                            op=ALU.mult)
                    pts.append((kb, off, pt))

                for h2 in range(2):
                    h = 2 * hp + h2
                    o_p = mm512.tile([128, 512], f32, tag="mm512")
                    for (kb, off, pt) in pts:
                        nc.tensor.matmul(
                            o_p[0:65, off:512],
                            v_sb[:, kb, h, 0:65],
                            pt[:, h2, off:512],
                            start=(kb == 0), stop=(kb == nkb - 1))
                    # den row 64 -> partition 0 (custom-DVE recip and
                    # partition_broadcast need partition-0 operands)
                    den64 = nrm.tile([65, 512], f32, tag="den64")
                    nc.vector.tensor_copy(den64[64:65, :], o_p[64:65, :])
                    den0 = nrm.tile([1, 512], f32, tag="den0")
                    nc.sync.dma_start(den0[:], den64[64:65, :])
                    rcp = nrm.tile([1, 512], f32, tag="rcp")
                    nc.vector.reciprocal_approx_fast(rcp[:], den0[:])
                    bc = nrm.tile([64, 512], f32, tag="bc")
                    nc.gpsimd.partition_broadcast(bc[:], rcp[:])
                    dest = (aT[hp][0:64, csl] if h2 == 0
                            else aTo[hp][:, csl])
                    nc.vector.tensor_tensor(dest, o_p[0:64, :], bc[:],
                                            op=ALU.mult)
            for hp in range(2):
                nc.gpsimd.dma_start(aT[hp][64:128, csl], aTo[hp][:, csl])

        def stage3(sc):
            scl = slice(sc * 512, (sc + 1) * 512)
            out_dst = outT_d.ap().rearrange("(a p) s -> p a s", a=8)
            po = po_pool.tile([128, 8, 512], bf16, tag="po")
            for dd in range(8):
                # evict on DVE during exp-heavy windows (sc<2) and for
                # alternate chunks at the tail, splitting work with ACT
                use_dve = (sc < 2) or (dd % 2 == 1)
                pp = mm512.tile([128, 512], f32, tag="mm512")
                for kc2 in range(2):
                    nc.tensor.matmul(
                        pp[:],
                        wp_sb[:, kc2, dd * 128:(dd + 1) * 128],
                        aT[kc2][:, scl],
                        start=(kc2 == 0), stop=(kc2 == 1) and not use_dve)
                if use_dve:
                    nc.tensor.matmul(
                        pp[:], bpT_sb[:, dd * 128:(dd + 1) * 128],
                        ones_row[:], start=False, stop=True)
                    nc.vector.tensor_copy(po[:, dd, :], pp[:])
                else:
                    nc.scalar.activation(po[:, dd, :], pp[:], AF.Identity,
                                         bias=bp_sb[:, dd:dd + 1])
                if sc == 3:
                    nc.sync.dma_start(out_dst[:, dd, scl], po[:, dd, :])
            if sc != 3:
                nc.sync.dma_start(out_dst[:, :, scl], po[:])

        # interleaved emission: keeps PE dense, lets exp start early;
        # proj chunks late so PE has fill work during the exp-bound tail
        stage1(0)
        stage1(1)
        stage2(0)
        stage1(2)
        stage2(1)
        stage1(3)
        stage2(2)
        stage3(0)
        stage3(1)
        stage2(3)
        stage3(2)
        stage3(3)

    nc.finalize()
    return nc


# pair-interleave: new row j within a head holds original dim PERM[j]
PERM = np.empty(64, np.int64)
PERM[0::2] = np.arange(32)
PERM[1::2] = np.arange(32) + 32


def make_core_inputs(inputs, core):
    """Host-side shard prep for one core."""
    import ml_dtypes
    bf = ml_dtypes.bfloat16
    b, g = core // 4, core % 4
    hidden = np.asarray(inputs["hidden_states"], dtype=np.float32)
    pos = np.asarray(inputs["position_ids"])
    caw = np.asarray(inputs["c_attn_w"], dtype=np.float32)
    cab = np.asarray(inputs["c_attn_b"], dtype=np.float32)
    cpw = np.asarray(inputs["c_proj_w"], dtype=np.float32)
    cpb = np.asarray(inputs["c_proj_b"], dtype=np.float32)

    cs = slice(g * HD, (g + 1) * HD)
    # per-head pair-interleaved column permutation for q and k
    hperm = np.concatenate([h * 64 + PERM for h in range(4)])
    wq = caw[:, cs][:, hperm]
    wk = caw[:, D + g * HD:D + (g + 1) * HD][:, hperm]
    wv = caw[:, 2 * D + g * HD:2 * D + (g + 1) * HD]
    wqkv = np.concatenate([wq, wk, wv], axis=1)

    bq = cab[cs][hperm]
    bk = cab[D + g * HD:D + (g + 1) * HD][hperm]
    bv = cab[2 * D + g * HD:2 * D + (g + 1) * HD]
    # bqk[:, jc]: jc0/1 = q head pairs, jc2/3 = k head pairs
    bqk = np.stack([bq[0:128], bq[128:256], bk[0:128], bk[128:256]],
                   axis=1).astype(np.float32)

    # rope tables in permuted transposed layout [128 rows = 2 heads x 64]
    inv_freq = (1.0 / (10000.0 **
                       (np.arange(0, 64, 2, dtype=np.float64) / 64.0)))
    theta = pos[b].astype(np.float64)[None, :] * inv_freq[:, None]  # [32,S]
    cosv = np.cos(theta)
    sinv = np.sin(theta)
    cos64 = np.empty((64, S), np.float64)
    sin64 = np.empty((64, S), np.float64)
    cos64[0::2] = cosv
    cos64[1::2] = cosv
    sin64[0::2] = -sinv      # row 2i   (orig dim i):    -sin
    sin64[1::2] = sinv       # row 2i+1 (orig dim i+32): +sin
    cosT = np.tile(cos64, (2, 1)).astype(bf)
    sinT = np.tile(sin64, (2, 1)).astype(bf)

    bp = (cpb if g == 0 else np.zeros_like(cpb)).reshape(8, 128).T.copy()

    r = np.arange(128)
    mask01 = (r[None, :] >= r[:, None]).astype(np.float32)
    mask2 = np.concatenate([mask01, mask01], axis=1)

    return {
        "hT": np.ascontiguousarray(hidden[b].T).astype(bf),
        "wqkv": np.ascontiguousarray(wqkv).astype(bf),
        "bqk": np.ascontiguousarray(bqk),
        "bv": bv[None, :].astype(bf),
        "bpT": bp.T.reshape(1, D).astype(bf),
        "cosT": cosT,
        "sinT": sinT,
        "wp": np.ascontiguousarray(cpw[cs, :]).astype(bf),
        "bp": np.ascontiguousarray(bp.astype(np.float32)),
        "mask2": mask2.astype(bf),
        "ones64": np.ones((128, 64), bf),
        "ones_row": np.ones((1, 512), bf),
    }


_NC_CACHE = {}


def run(inputs, trace=False, **spmd_kwargs):
    """Shard, execute on 8 cores, unshard. Returns (output, BassKernelResults)."""
    if "nc" not in _NC_CACHE:
        _NC_CACHE["nc"] = build_attention_nc(num_devices=8)
    nc = _NC_CACHE["nc"]
    in_maps = [make_core_inputs(inputs, c) for c in range(8)]
    res = run_bass_kernel_spmd(nc, in_maps, core_ids=list(range(8)),
                               trace=trace, **spmd_kwargs)
    outs = []
    for b in range(2):
        acc = np.zeros((D, S), np.float64)
        for g in range(4):
            acc += res.results[b * 4 + g]["outT"].astype(np.float64)
        outs.append(acc.T.astype(np.float32))
    return np.stack(outs, axis=0), res


def kernel(**inputs) -> np.ndarray:
    out, _ = run(inputs, trace=False)
    return out
